# revision 61
# baseline (speedup 1.0000x reference)
"""Trainium2 Bass kernel for multi-head causal attention with RoPE.

Problem: x[4,2048,1024] -> MHA(16 heads, head_dim 64, RoPE, causal) -> [4,2048,1024]

Sharding: 8 cores = 4 batches x 2 head-groups (8 heads each, Megatron-style).
Each core computes a partial [T, C] projection output for its batch; the host
sums the two head-group partials per batch and adds b_proj.

Per-core dataflow, chunked by 512-row t-blocks:
  A(tcn): x^T via DMA-engine xbar transposes (HBM -> SBUF, zero PE/DVE cost),
          Q^T/K^T bf16 GEMMs into a [2 heads x 64d] row layout with RoPE via
          a rot-half permutation matmul + elementwise muls; V in [t, h*64+e]
          bf16 with the qkv bias fused into the PSUM->SBUF copy
  B(qc=tcn): per head-pair (2g, 2g+1): scores S^T = K Q^T (bf16, K=64);
          causal masking of diagonal blocks by a -64 triangular matmul
          accumulated into the score PSUM pre-exp (no vector mask work);
          exp on ACT to bf16; PV flipped (P^T stationary, V moving) so all
          128 output partitions are useful y[q, e]; softmax denominator via
          a ones-column matmul; accumulation groups share a PSUM bank, so
          banks are pre-zeroed and all PV matmuls run with start=False
          (a start=True would mark the whole 2KB bank pending-zero and
          clobber sibling groups); normalize = one reciprocal + broadcast
          multiply per head pair; y -> y^T via one SBUF xbar-transpose DMA
          per chunk
  C(tcn): y^T @ W_proj (bf16), one batched out DMA per chunk

DMA instruction count is minimized (19 total): in this cost model each HWDGE
DMA serializes globally at ~3us (dge delay + transfer + sem propagation), so
per-chunk batching of the x-transpose, y-transpose and output store matters
more than transfer size. Weight loads are ordered by first use so the PE can
start ~6us in; chunk-0's slot-1 QK weights ride the b_phase(0) feed.
"""

import math
import sys

import numpy as np

if "/opt/trn_rl_repo" not in sys.path:
    sys.path.insert(0, "/opt/trn_rl_repo")

import concourse.bass as bass
import concourse.tile as tile
from concourse import bacc
from concourse import mybir
from concourse.bass_utils import run_bass_kernel_spmd
from concourse.masks import make_identity

B, T, C = 4, 2048, 1024
NH, D = 16, 64
HL = 8              # local heads per core
DL = HL * D         # 512
NCORES = 8
P = 128
TCH = 512           # t-chunk width
NTC = T // TCH
ROPE_BASE = 10000.0

F32 = mybir.dt.float32
BF16 = mybir.dt.bfloat16
Exp = mybir.ActivationFunctionType.Exp
Mul = mybir.AluOpType.mult
Add = mybir.AluOpType.add


def _emit(tc, xb, wqk, wv, wp, cs, bias, tri, perm, out, dbg=None):
    nc = tc.nc
    with tc.tile_pool(name="pers", bufs=1) as pers:
        wqk_sb = pers.tile([P, 8, 8, P], BF16)    # [p, j, cc, n]
        wv_sb = pers.tile([P, 8, DL], BF16)       # [p, cc, h*64+e]
        wp_sb = pers.tile([P, 4, C], BF16)        # [e2, g, n]
        cs_sb = pers.tile([P, 2, T], BF16)        # cos/sin, row r -> freq r%32
        bias_sb = pers.tile([P, 8 + DL], F32)
        tri_sb = pers.tile([P, P], BF16)          # tri[c,k] = -64*[c<k]
        perm_sb = pers.tile([P, P], BF16)         # rot-half permutation
        ident = pers.tile([P, P], BF16)
        make_identity(nc, ident)
        oneb = pers.tile([P, 1], BF16)
        nc.vector.memset(oneb[:], 1.0)
        # Q^T/K^T bf16: [row = h2*64 + d, j = 2g + kind (Q/K of pair g), t]
        qkb = pers.tile([P, 8, T], BF16)
        # V bf16: [t%128, t-tile, head, e | ones]; col 64 feeds the
        # softmax denominator through the same PV matmul
        vsb = pers.tile([P, 16, HL, D + 1], BF16)
        nc.vector.memset(vsb.rearrange("p a b c -> p (a b) c")[:, :, D:], 1.0)
        bias_v = bias_sb[:, 8:].rearrange("p (h e) -> p h e", e=D)

        # first-use-ordered weight loads; xT chunk-0 transpose interleaves
        nc.scalar.dma_start(wqk_sb[:, 0:1],
                            wqk[0:1].rearrange("j p cc n -> p j cc n"))

        with tc.tile_pool(name="xT", bufs=3) as pxT, \
             tc.tile_pool(name="t1", bufs=3) as pt1, \
             tc.tile_pool(name="tmp", bufs=4) as ptmp, \
             tc.tile_pool(name="pt", bufs=8) as ppt, \
             tc.tile_pool(name="y", bufs=2) as py, \
             tc.tile_pool(name="rcp", bufs=2) as prcp, \
             tc.tile_pool(name="yT", bufs=2) as pyT, \
             tc.tile_pool(name="ost", bufs=2) as post, \
             tc.tile_pool(name="psS", bufs=2, space="PSUM") as psS, \
             tc.tile_pool(name="psQ", bufs=2, space="PSUM") as psQ, \
             tc.tile_pool(name="psO", bufs=2, space="PSUM") as psO_p:

            def a_units(tcn):
                """Chunk tcn's QKV phase: xbar-transpose DMA, QK GEMM+RoPE,
                V GEMM. Units sized ~0.9us for fine interleaving."""
                ts0 = tcn * TCH
                xT = pxT.tile([P, 8, TCH], BF16, tag="xT")
                units = []

                def xt_unit(xT=xT, ts0=ts0):
                    nc.sync.dma_start_transpose(
                        xT[:], xb[ts0: ts0 + TCH, :])
                xt_list = [xt_unit]

                qk_st = {}

                def qk_half_a(j, xT=xT):
                    psq = psQ.tile([P, TCH], F32, tag="q")
                    qk_st[j] = psq
                    for cc in range(4):
                        nc.tensor.matmul(
                            psq[:],
                            wqk_sb[:, j, cc, :],
                            xT[:, cc, :],
                            start=(cc == 0), stop=False)

                def qk_unit(j, xT=xT, ts0=ts0):
                    psq = qk_st.pop(j)
                    for cc in range(4, 8):
                        nc.tensor.matmul(
                            psq[:],
                            wqk_sb[:, j, cc, :],
                            xT[:, cc, :],
                            start=False, stop=(cc == 7))
                    t1 = pt1.tile([P, TCH], BF16, tag="t1")
                    nc.vector.tensor_scalar_add(t1[:], psq[:],
                                                bias_sb[:, j:j + 1])
                    # psq is dead after the bias copy; reuse its bank for
                    # the rot-half permutation product (keeps psQ at one
                    # allocation per unit so the 2-buf ring never wraps
                    # onto a live tile)
                    nc.tensor.matmul(psq[:], perm_sb[:], t1[:],
                                     start=True, stop=True,
                                     skip_group_check=True)
                    dst = qkb[:, j, ts0:ts0 + TCH]
                    nc.vector.tensor_tensor(dst, t1[:],
                                            cs_sb[:, 0, ts0:ts0 + TCH], Mul)
                    swp = ptmp.tile([P, TCH], BF16, tag="tmp")
                    nc.vector.tensor_tensor(swp[:], psq[:],
                                            cs_sb[:, 1, ts0:ts0 + TCH], Mul)
                    nc.vector.tensor_tensor(dst, dst, swp[:], Add)
                for j in range(4):
                    units.append(lambda j=j: qk_half_a(j))
                    units.append(lambda j=j: qk_unit(j))

                def v_half_a(i, xT=xT):
                    psv = psQ.tile([P, DL], F32, tag="q")
                    qk_st[8 + i] = psv
                    for cc in range(4):
                        nc.tensor.matmul(
                            psv[:],
                            xT[:, cc, i * P:(i + 1) * P],
                            wv_sb[:, cc, :],
                            start=(cc == 0), stop=False)

                def v_unit(i, xT=xT, tcn=tcn):
                    ti = tcn * (TCH // P) + i
                    psv = qk_st.pop(8 + i)
                    for cc in range(4, 8):
                        nc.tensor.matmul(
                            psv[:],
                            xT[:, cc, i * P:(i + 1) * P],
                            wv_sb[:, cc, :],
                            start=False, stop=(cc == 7))
                    psvh = psv.rearrange("p (h e) -> p h e", e=D)
                    nc.vector.tensor_tensor(vsb[:, ti, :, 0:D], psvh,
                                            bias_v, Add)
                for i in range(TCH // P):
                    units.append(lambda i=i: v_half_a(i))
                    units.append(lambda i=i: v_unit(i))
                for j in range(4, 8):
                    units.append(lambda j=j: qk_half_a(j))
                    units.append(lambda j=j: qk_unit(j))
                return xt_list, units

            def c_units(tcn, yTt):
                """Projection for chunk tcn; needs yTt complete."""
                ts0 = tcn * TCH
                units = []

                ost = post.tile([P, 4, C], BF16, tag="ost",
                                name=f"ost_{tcn}")

                def c_unit(i, n, yTt=yTt, ost=ost):
                    psp = psQ.tile([P, 512], F32, tag="q")
                    for g in range(4):
                        nc.tensor.matmul(
                            psp[:],
                            yTt[:, g, i * P:(i + 1) * P],
                            wp_sb[:, g, n * 512:(n + 1) * 512],
                            start=(g == 0), stop=(g == 3))
                    nc.vector.tensor_copy(
                        ost[:, i, n * 512:(n + 1) * 512], psp[:])

                def c_flush_half(h, ost=ost, ts0=ts0):
                    nc.sync.dma_start(
                        out[ts0 + h * 256: ts0 + (h + 1) * 256, :].rearrange(
                            "(i p) c -> p i c", p=P), ost[:, 2 * h:2 * h + 2])

                def c_flush(ost=ost, ts0=ts0):
                    nc.sync.dma_start(
                        out[ts0: ts0 + TCH, :].rearrange(
                            "(i p) c -> p i c", p=P), ost[:])
                last = (tcn == NTC - 1)
                for i in range(TCH // P):
                    for n in range(2):
                        units.append(lambda i=i, n=n: c_unit(i, n))
                        if last and i == 1 and n == 1:
                            units.append(lambda: c_flush_half(0))
                if last:
                    units.append(lambda: c_flush_half(1))
                else:
                    units.append(c_flush)
                return units

            def b_phase(qc, feed, pre):
                """Attention for q-chunk qc; drains `feed` units into PE
                slack while ACT exps. `pre` = next chunk's x-transpose
                (urgent, wait-free) then the previous chunk's y->yT
                transpose."""
                for f in pre:
                    f()
                nblk = 4 * qc + 4
                yTt = pyT.tile([P, 4, TCH], BF16, tag="yT")
                ych = py.tile([P, 4, 4, P], BF16, tag="y",
                              name=f"ych_{qc}")
                drain = {"done": 0, "seen": 0, "n0": max(1, len(feed))}
                total_kc = 4 * nblk

                def drain_tick():
                    # spread the feed evenly over the phase's kc steps,
                    # holding back a few units to cover the PV flushes
                    drain["seen"] += 1
                    want = min(drain["seen"] * drain["n0"] // total_kc,
                               max(0, drain["n0"] - 4))
                    while feed and drain["done"] < want:
                        feed.pop(0)()
                        drain["done"] += 1
                for g in range(4):
                    psO0 = psO_p.tile([P, 4, D + 1], F32, tag="o",
                                      name=f"psO0_{qc}_{g}")
                    psO1 = psO_p.tile([P, 4, D + 1], F32, tag="o",
                                      name=f"psO1_{qc}_{g}")
                    psO = (psO0, psO1)
                    # pre-zero: 4 accumulation groups share each bank; a
                    # start=True would mark the whole 2KB bank pending-zero
                    # and clobber sibling groups, so accumulate-only.
                    nc.vector.memset(psO0[:], 0.0)
                    nc.vector.memset(psO1[:], 0.0)
                    pv_q = []          # (kc, pt) with 3-block lag

                    def pv_blk(kc, pt, qc=qc, g=g, psO=psO):
                        for hh in range(2):
                            head = 2 * g + hh
                            for qi in range(4):
                                last_kc = 4 * qc + qi
                                if kc > last_kc:
                                    continue
                                stop = (kc == last_kc)
                                lhs = pt[:, hh * 512 + qi * P:
                                         hh * 512 + (qi + 1) * P]
                                nc.tensor.matmul(
                                    psO[hh][:, qi, :], lhs,
                                    vsb[:, kc, head, :],
                                    start=False, stop=stop,
                                    skip_group_check=True)

                    for kc in range(nblk):
                        if qc == 0 and feed and kc < 2:
                            feed.pop(0)()     # chunk-0 carry must emit early
                        elif kc >= 2:
                            drain_tick()
                        m = kc - 4 * qc
                        pt = ppt.tile([P, 1024], BF16, tag="pt",
                                      name=f"pt_{qc}_{g}_{kc}")
                        pss = psS.tile([P, 1024], F32, tag="s")
                        for hh in range(2):
                            q0 = m * P if m > 0 else 0
                            nc.tensor.matmul(
                                pss[:, hh * 512 + q0:(hh + 1) * 512],
                                qkb[64 * hh:64 * hh + 64, 2 * g + 1,
                                    kc * P:(kc + 1) * P],
                                qkb[64 * hh:64 * hh + 64, 2 * g,
                                    qc * TCH + q0:(qc + 1) * TCH],
                                start=True, stop=(m < 0),
                                skip_group_check=True,
                                tile_position=(64 * hh, 0))
                            if m >= 0:
                                nc.tensor.matmul(
                                    pss[:, hh * 512 + q0:
                                        hh * 512 + q0 + P],
                                    tri_sb[:], ident[:],
                                    start=False, stop=True,
                                    skip_group_check=True)
                        with nc.allow_low_precision(reason="bf16 softmax"):
                            if m < 0:
                                nc.scalar.activation(pt[:], pss[:], Exp)
                            else:
                                q0 = m * P
                                nc.scalar.activation(
                                    pt.rearrange("p (h w) -> p h w",
                                                 h=2)[:, :, q0:],
                                    pss.rearrange("p (h w) -> p h w",
                                                  h=2)[:, :, q0:],
                                    Exp)
                        if len(pv_q) >= 4:
                            pv_blk(*pv_q.pop(0))
                        pv_q.append((kc, pt))
                    while pv_q:
                        pv_blk(*pv_q.pop(0))
                        if feed and len(pv_q) % 2 == 1:
                            feed.pop(0)()

                    rcp = prcp.tile([P, 2, 4], F32, tag="rcp")
                    yv = ych[:, g].rearrange("p a (h e) -> p h a e", h=2)
                    for hh in range(2):
                        nc.vector.reciprocal(rcp[:, hh],
                                             psO[hh][:, :, D])
                        nc.vector.tensor_tensor(
                            yv[:, hh], psO[hh][:, :, 0:D],
                            rcp[:, hh].rearrange(
                                "p (a o) -> p a o", o=1).to_broadcast(
                                    (P, 4, D)),
                            Mul)
                    if feed and g < 3:
                        feed.pop(0)()
                    if qc == NTC - 1 and g == 1:
                        # last chunk: transpose the first half early so the
                        # final projection's g0/g1 matmuls start immediately
                        nc.sync.dma_start_transpose(
                            yTt[:, 0:2].rearrange(
                                "p g (a c) -> p (g a) c", c=P),
                            ych[:, 0:2])
                while feed:
                    feed.pop(0)()

                if qc == NTC - 1:
                    def ytr(ych=ych, yTt=yTt):
                        nc.sync.dma_start_transpose(
                            yTt[:, 2:4].rearrange(
                                "p g (a c) -> p (g a) c", c=P),
                            ych[:, 2:4])
                else:
                    def ytr(ych=ych, yTt=yTt):
                        nc.sync.dma_start_transpose(
                            yTt.rearrange("p g (a c) -> p (g a) c", c=P),
                            ych[:])
                return yTt, [ytr]

            xt0, a0 = a_units(0)
            nc.scalar.dma_start(bias_sb[:], bias)
            nc.scalar.dma_start(tri_sb[:], tri)
            nc.scalar.dma_start(perm_sb[:], perm)
            for u in xt0:              # x^T chunk-0 transpose (SP queue)
                u()
            nc.scalar.dma_start(wqk_sb[:, 1:2],
                                wqk[1:2].rearrange("j p cc n -> p j cc n"))
            nc.scalar.dma_start(cs_sb[:], cs)
            nc.scalar.dma_start(wqk_sb[:, 2:4],
                                wqk[2:4].rearrange("j p cc n -> p j cc n"))
            for u in a0[:8]:           # qk of head-pairs 0,1 (j=0..3)
                u()
            nc.sync.dma_start(wv_sb[:], wv)
            nc.scalar.dma_start(wqk_sb[:, 4:8],
                                wqk[4:8].rearrange("j p cc n -> p j cc n"))
            nc.sync.dma_start(wp_sb[:], wp)
            for u in a0[8:16]:         # v units (needed by b_phase(0) PV)
                u()
            # chunk-0 QK of head-pairs 2,3 ride as b_phase(0) feed: their
            # weights (wqk[4:8]) land late in the DMA chain and must not
            # block the first head-pairs' scores in PE program order; they
            # drain before g=2 needs them
            yT_prev, ytr_prev = None, []
            carry = a0[16:]
            for tcn in range(NTC):
                feed = list(carry)
                carry = []
                pre = []
                if yT_prev is not None:
                    feed.extend(c_units(tcn - 1, yT_prev))
                if tcn + 1 < NTC:
                    xt_n, a_n = a_units(tcn + 1)
                    pre.extend(xt_n)
                    feed.extend(a_n)
                pre.extend(ytr_prev)
                yT_prev, ytr_prev = b_phase(tcn, feed, pre)
            for f in ytr_prev:
                f()
            for u in c_units(NTC - 1, yT_prev):
                u()
            if dbg is not None:
                nc.sync.dma_start(dbg["qkb"], qkb[:])
                nc.sync.dma_start(dbg["vsb"], vsb[:])
                nc.sync.dma_start(dbg["yT3"], yT_prev[:])


def build_nc(debug=False):
    nc = bacc.Bacc("TRN2", target_bir_lowering=False, debug=False)
    xb = nc.dram_tensor("xb", [T, C], BF16, kind="ExternalInput").ap()
    wqk = nc.dram_tensor("wqk", [8, P, 8, P], BF16, kind="ExternalInput").ap()
    wv = nc.dram_tensor("wv", [P, 8, DL], BF16, kind="ExternalInput").ap()
    wp = nc.dram_tensor("wp", [P, 4, C], BF16, kind="ExternalInput").ap()
    cs = nc.dram_tensor("cs", [P, 2, T], BF16, kind="ExternalInput").ap()
    bias = nc.dram_tensor("bias", [P, 8 + DL], F32, kind="ExternalInput").ap()
    tri = nc.dram_tensor("tri", [P, P], BF16, kind="ExternalInput").ap()
    perm = nc.dram_tensor("perm", [P, P], BF16, kind="ExternalInput").ap()
    out = nc.dram_tensor("out", [T, C], BF16, kind="ExternalOutput").ap()
    dbg = None
    if debug:
        dbg = {
            "qkb": nc.dram_tensor("d_qkb", [P, 8, T], BF16,
                                  kind="ExternalOutput").ap(),
            "vsb": nc.dram_tensor("d_vsb", [P, 16, HL, D], BF16,
                                  kind="ExternalOutput").ap(),
            "yT3": nc.dram_tensor("d_yT3", [P, 4, TCH], BF16,
                                  kind="ExternalOutput").ap(),
        }
    with tile.TileContext(nc) as tc:
        _emit(tc, xb, wqk, wv, wp, cs, bias, tri, perm, out, dbg=dbg)
    nc.compile()
    return nc


def rope_tables():
    inv_freq = 1.0 / (ROPE_BASE ** (np.arange(0, D, 2, dtype=np.float64) / D))
    t = np.arange(T, dtype=np.float64)
    freqs = np.outer(t, inv_freq)                      # [T, 32]
    cosT = np.cos(freqs).T.astype(np.float32)          # [32, T]
    sinT = np.sin(freqs).T.astype(np.float32)
    cos4 = np.tile(cosT, (4, 1))                       # [128, T]
    sin4 = np.tile(sinT, (4, 1))
    return np.ascontiguousarray(np.stack([cos4, sin4], axis=1))  # [128,2,T]


def perm_matrix():
    pm = np.zeros((P, P), dtype=np.float32)
    for base in (0, 64):
        for d in range(32):
            pm[base + d + 32, base + d] = -1.0       # rot_half: -x2 into top
            pm[base + d, base + d + 32] = 1.0        # +x1 into bottom
    return pm


def host_inputs(x, W_qkv, b_qkv, W_proj, b_proj):
    import ml_dtypes
    bf16 = ml_dtypes.bfloat16
    x = np.asarray(x, dtype=np.float32)
    W_qkv = np.asarray(W_qkv, dtype=np.float32)
    b_qkv = np.asarray(b_qkv, dtype=np.float32)
    W_proj = np.asarray(W_proj, dtype=np.float32)
    scale = 1.0 / math.sqrt(D)
    cs = rope_tables().astype(bf16)
    tri = np.zeros((P, P), dtype=np.float32)
    for c_ in range(P):
        tri[c_, c_ + 1:] = -64.0
    tri = np.ascontiguousarray(tri.astype(bf16))
    pm = np.ascontiguousarray(perm_matrix().astype(bf16))

    in_maps = []
    for core in range(NCORES):
        b = core // 2
        hg = core % 2
        s = hg * DL
        # wqk: [j, p, cc, n]; j = 2g + kind; psq row n = h2*64 + d;
        # W col = kind*C + s + (2g + h2)*64 + d ; input channel = cc*128+p
        cols = np.empty((8, P), dtype=np.int64)
        for j in range(8):
            g, kind = j // 2, j % 2
            for n in range(P):
                h2, d = n // 64, n % 64
                cols[j, n] = kind * C + s + (2 * g + h2) * 64 + d
        wqk_d = np.empty((8, P, 8, P), dtype=np.float32)
        for j in range(8):
            wcols = W_qkv[:, cols[j]]                    # [1024, 128]
            if j % 2 == 0:                               # Q: fold 1/sqrt(D)
                wcols = wcols * scale
            wqk_d[j] = wcols.reshape(8, P, P).transpose(1, 0, 2)
        wqk_d = np.ascontiguousarray(wqk_d.astype(bf16))
        wv_f = W_qkv[:, 2 * C + s:2 * C + s + DL]        # [1024, 512]
        wv_d = np.ascontiguousarray(
            wv_f.reshape(8, P, DL).transpose(1, 0, 2).astype(bf16))
        # wp rows: e2-row p of g-tile = head 2g + p//64, e = p%64
        wp_d = np.empty((P, 4, C), dtype=np.float32)
        for g in range(4):
            for p_ in range(P):
                head = 2 * g + (p_ // 64)
                wp_d[p_, g] = W_proj[s + head * 64 + (p_ % 64), :]
        wp_d = np.ascontiguousarray(wp_d.astype(bf16))
        bias_d = np.zeros((P, 8 + DL), dtype=np.float32)
        for j in range(8):
            bias_d[:, j] = b_qkv[cols[j]]
            if j % 2 == 0:
                bias_d[:, j] *= scale
        bias_d[:, 8:] = np.tile(b_qkv[2 * C + s:2 * C + s + DL][None, :],
                                (P, 1))
        in_maps.append({
            "xb": np.ascontiguousarray(x[b].astype(bf16)),
            "wqk": wqk_d, "wv": wv_d, "wp": wp_d,
            "cs": cs, "bias": np.ascontiguousarray(bias_d), "tri": tri,
            "perm": pm,
        })
    return in_maps


_NC_CACHE = {}


def run(in_maps, **kwargs):
    if "nc" not in _NC_CACHE:
        _NC_CACHE["nc"] = build_nc()
    return run_bass_kernel_spmd(
        _NC_CACHE["nc"], in_maps, core_ids=list(range(NCORES)), **kwargs)


def kernel(x, W_qkv, b_qkv, W_proj, b_proj, **extra):
    in_maps = host_inputs(x, W_qkv, b_qkv, W_proj, b_proj)
    res = run(in_maps)
    b_proj = np.asarray(b_proj, dtype=np.float32)
    out = np.empty((B, T, C), dtype=np.float32)
    for b in range(B):
        out[b] = (res.results[2 * b]["out"].astype(np.float32)
                  + res.results[2 * b + 1]["out"].astype(np.float32) + b_proj)
    return out
